# revision 32
# baseline (speedup 1.0000x reference)
"""Sliding-window GQA attention (B=1, S=4096, HID=1024, H=16, KV=4, D=64, W=512)
sharded across 8 trn2 NeuronCores by sequence (512 query rows/core + 512-row
k/v halo recomputed locally; core 0's halo is zero-padded and its softmax
denominator corrected through the sink term).

Self-contained: takes full inputs, shards on host, runs one SPMD Bass kernel
on cores 0-7, reassembles the full output.
"""
import sys
sys.path.insert(0, '/opt/trn_rl_repo')
import numpy as np
import ml_dtypes  # noqa: F401  (registers bfloat16 with numpy)
BF16NP = np.dtype('bfloat16')

import concourse.bass as bass
import concourse.bacc as bacc
import concourse.hw_specs as _hw_specs

# Route every Ln/Exp activation to the single set that contains both
# ("natural_log_exp_and_others"), so the scheduler's interleaving of Ln and
# Exp ops never forces an ACT table reload (~2.7us each).
_orig_get_act_tables = _hw_specs.get_activation_tables


def _merged_act_tables(arch):
    t = dict(_orig_get_act_tables(arch))
    strip = {mybir.ActivationFunctionType.Ln, mybir.ActivationFunctionType.Exp,
             mybir.ActivationFunctionType.Square}
    for name, fns in t.items():
        if name != "natural_log_exp_and_others":
            t[name] = fns - strip
    return t


bacc.get_activation_tables = _merged_act_tables
import concourse.tile as tile
import concourse.mybir as mybir
from concourse.bass_utils import run_bass_kernel_spmd

F32 = mybir.dt.float32
F32R = mybir.dt.float32r
BF16 = mybir.dt.bfloat16
AF = mybir.ActivationFunctionType
OP = mybir.AluOpType

B, S, HID = 1, 4096, 1024
H, KV, D = 16, 4, 64
WINDOW = 512
EPS = 1e-5
NEG = -1e9
SCALE = 0.125  # 1/sqrt(D)
NCORE = 8
SLOC = 512    # query rows per core
SKV = 1024    # k/v rows per core (halo + own)

_cache = {}


def _build(phases="ABC"):
    nc = bacc.Bacc("TRN2", target_bir_lowering=False, debug=False, num_devices=NCORE)

    xT = nc.dram_tensor("xT", [HID, SKV], BF16, kind="ExternalInput").ap()
    wqT = nc.dram_tensor("wqT", [HID, HID], BF16, kind="ExternalInput").ap()
    wkvT = nc.dram_tensor("wkvT", [HID, 512], BF16, kind="ExternalInput").ap()
    woT = nc.dram_tensor("woT", [HID, HID], BF16, kind="ExternalInput").ap()
    cqsqd = nc.dram_tensor("cqsq", [128, 512], F32, kind="ExternalInput").ap()
    ckskd = nc.dram_tensor("cksk", [128, 1024], F32, kind="ExternalInput").ap()
    maskd = nc.dram_tensor("masks", [128, 4 * 256], BF16, kind="ExternalInput").ap()
    identd = nc.dram_tensor("ident", [128, 128], BF16, kind="ExternalInput").ap()
    identrd = nc.dram_tensor("identr", [128, 128], F32R, kind="ExternalInput").ap()
    sinkd = nc.dram_tensor("sink_rhs", [16, 512], F32R, kind="ExternalInput").ap()
    vonesd = nc.dram_tensor("vones", [SKV, 1], BF16, kind="ExternalInput").ap()
    outd = nc.dram_tensor("out", [SLOC, HID], F32, kind="ExternalOutput").ap()

    with tile.TileContext(nc) as tc:
        with tc.tile_pool(name="const", bufs=1) as cpool, \
             tc.tile_pool(name="persist", bufs=1) as pers:

            # ---------------- tiny constants (first: cheap, needed early) ----
            ident = cpool.tile([128, 128], BF16, tag="ident")
            nc.gpsimd.dma_start(ident[:], identd)
            identr = cpool.tile([128, 128], F32R, tag="identr")
            nc.gpsimd.dma_start(identr[:], identrd)
            epsc = cpool.tile([128, 1], F32, tag="epsc")
            nc.vector.memset(epsc[:], EPS)

            # persistent products of phase A
            v_sb = [pers.tile([128, 260], BF16, tag=f"v{st}", name=f"v{st}") for st in range(8)]
            qT_sb = [pers.tile([128, 512], F32R, tag=f"qT{db}", name=f"qT{db}") for db in range(8)]
            kTd_sb = [pers.tile([128, SKV], F32R, tag=f"kT{kv}", name=f"kT{kv}") for kv in range(4)]
            aoT_sb = [pers.tile([128, 512], BF16, tag=f"aoT{p}", name=f"aoT{p}") for p in range(8)]

            with tc.tile_pool(name="wkp", bufs=1) as wkp:
                # ---- input DMAs, ordered by first use: x/wkv interleaved,
                # then rope-k, then rest of x, then cq/sq + wq, masks last.
                xT_sb = [None] * 8
                wkv_sb = [None] * 8

                def load_x(kb):
                    t = pers.tile([128, SKV], BF16, tag=f"xT{kb}", name=f"xT{kb}")
                    eng = nc.sync if kb % 2 == 0 else nc.scalar
                    eng.dma_start(t[:], xT[kb * 128:(kb + 1) * 128, :])
                    xT_sb[kb] = t

                def load_wkv(kb):
                    t = wkp.tile([128, 512], BF16, tag=f"wkv{kb}", name=f"wkv{kb}")
                    eng = nc.scalar if kb % 2 == 0 else nc.sync
                    eng.dma_start(t[:], wkvT[kb * 128:(kb + 1) * 128, :])
                    wkv_sb[kb] = t

                for kb in range(4):
                    load_x(kb)
                    load_wkv(kb)
                cksk_sb = wkp.tile([128, 1024], F32, tag="cksk")
                nc.sync.dma_start(cksk_sb[:], ckskd)
                for kb in range(4, 8):
                    load_x(kb)
                    load_wkv(kb)
                cqsq_sb = pers.tile([128, 512], F32, tag="cqsq")
                nc.sync.dma_start(cqsq_sb[:], cqsqd)
                wq_sb = []

                def load_wq():
                    for kb in range(8):
                        t = pers.tile([128, HID], BF16, tag=f"wq{kb}", name=f"wq{kb}")
                        nc.gpsimd.dma_start(t[:], wqT[kb * 128:(kb + 1) * 128, :])
                        wq_sb.append(t)

                # ======== phase A2: k/v projection + norm + rope + transpose ====
                with tc.tile_pool(name="tAk", bufs=3) as tA, \
                     tc.tile_pool(name="psmmk", bufs=4, space="PSUM") as psmm, \
                     tc.tile_pool(name="pstk", bufs=1, space="PSUM") as pst:

                    def k_rope(st, kvp):
                        # rope applied to the raw projection; the rms scale is
                        # per-(row,head) so it commutes to after the rotation
                        k1 = tA.tile([128, 256], F32, tag="k1", name=f"k1_{st}")
                        k1v = k1[:].rearrange("p (h d) -> p h d", h=KV)
                        kpv = kvp[:, 0:256].rearrange("p (h d) -> p h d", h=KV)
                        ck = cksk_sb[:, st * 64:(st + 1) * 64]
                        sk = cksk_sb[:, 512 + st * 64: 512 + (st + 1) * 64]
                        nc.vector.tensor_mul(
                            k1v, kpv, ck.unsqueeze(1).broadcast_to([128, KV, D]))
                        k2 = tA.tile([128, 256], F32, tag="k2", name=f"k2_{st}")
                        nc.vector.tensor_mul(
                            k2[:].rearrange("p (h a j) -> p h a j", h=KV, a=2),
                            kpv.rearrange("p h (a j) -> p h a j", a=2)[:, :, ::-1, :],
                            sk.rearrange("p (a j) -> p a j", a=2)
                            .unsqueeze(1).broadcast_to([128, KV, 2, 32]))
                        nc.vector.tensor_add(k1[:], k1[:], k2[:])
                        return k1

                    def k_stage2(st, kr, msk, kTps):
                        lnk = tA.tile([128, KV], F32, tag="lnk", name=f"lnk{st}")
                        nc.scalar.activation(lnk[:], msk[:], AF.Ln, bias=epsc[:],
                                             scale=1.0 / D)
                        invk = tA.tile([128, KV], F32, tag="invk", name=f"invk{st}")
                        nc.scalar.activation(invk[:], lnk[:], AF.Exp, scale=-0.5)
                        krs = tA.tile([128, 256], F32R, tag="krs", name=f"krs{st}")
                        nc.vector.tensor_mul(
                            krs[:].rearrange("p (h d) -> p h d", h=KV),
                            kr[:].rearrange("p (h d) -> p h d", h=KV),
                            invk[:].unsqueeze(2).broadcast_to([128, KV, D]))
                        j = st % 4
                        for kv in range(4):
                            sl = slice(kv * 64, (kv + 1) * 64)
                            nc.tensor.matmul(kTps[kv][:, j * 128:(j + 1) * 128],
                                             krs[:, sl], identr[:],
                                             is_transpose=True, start=True, stop=True)

                    def k_stage1(st):
                        kvp = psmm.tile([128, 512], F32, tag="kvp", name=f"kvp{st}")
                        for kb in range(8):
                            nc.tensor.matmul(kvp[:], xT_sb[kb][:, st * 128:(st + 1) * 128],
                                             wkv_sb[kb][:], start=(kb == 0), stop=(kb == 7))
                        nc.scalar.copy(
                            v_sb[st][:].rearrange("p (h d) -> p h d", d=65)[:, :, 0:64],
                            kvp[:, 256:512].rearrange("p (h d) -> p h d", d=64))
                        nc.gpsimd.dma_start(
                            v_sb[st][:].rearrange("p (h d) -> p h d", d=65)[:, :, 64:65],
                            vonesd[st * 128:(st + 1) * 128, 0:1].unsqueeze(1)
                            .broadcast_to([128, KV, 1]))
                        sqk = tA.tile([128, 256], F32, tag="sqk", name=f"sqk{st}")
                        nc.scalar.activation(sqk[:], kvp[:, 0:256], AF.Square)
                        msk = tA.tile([128, KV], F32, tag="msk", name=f"msk{st}")
                        nc.vector.tensor_reduce(
                            msk[:], sqk[:].rearrange("p (h d) -> p h d", h=KV),
                            axis=mybir.AxisListType.X, op=OP.add)
                        kr = k_rope(st, kvp)
                        return kr, msk

                    for sh in range(2):
                        if sh == 1:
                            load_wq()
                        kTps = [pst.tile([64, 512], F32R, tag=f"kTps{kv}",
                                         name=f"kTps{sh}_{kv}") for kv in range(4)]
                        pend = []
                        for st4 in range(4):
                            st = sh * 4 + st4
                            pend.append((st, *k_stage1(st)))
                            if len(pend) > 2:
                                k_stage2(*pend.pop(0), kTps)
                        for it in pend:
                            k_stage2(*it, kTps)
                        for kv in range(4):
                            nc.scalar.copy(
                                kTd_sb[kv][0:64, sh * 512:(sh + 1) * 512], kTps[kv][:])
                            nc.gpsimd.dma_start(kTd_sb[kv][64:128, sh * 512:(sh + 1) * 512],
                                                kTd_sb[kv][0:64, sh * 512:(sh + 1) * 512])

                # ======== phase A1: q projection + norm + rope + transpose ======
                def emit_A1(hf, tA, psmm, pst):

                    def q_rope(st, hf, qp):
                        q1 = tA.tile([128, 512], F32, tag="q1", name=f"q1_{st}{hf}")
                        q1v = q1[:].rearrange("p (h d) -> p h d", h=8)
                        qpv = qp[:].rearrange("p (h d) -> p h d", h=8)
                        i = st - 4
                        ct = cqsq_sb[:, i * 64:(i + 1) * 64]
                        stt = cqsq_sb[:, 256 + i * 64: 256 + (i + 1) * 64]
                        nc.vector.tensor_mul(
                            q1v, qpv, ct.unsqueeze(1).broadcast_to([128, 8, D]))
                        q2 = tA.tile([128, 512], F32, tag="q2", name=f"q2_{st}{hf}")
                        nc.vector.tensor_mul(
                            q2[:].rearrange("p (h a j) -> p h a j", h=8, a=2),
                            qpv.rearrange("p h (a j) -> p h a j", a=2)[:, :, ::-1, :],
                            stt.rearrange("p (a j) -> p a j", a=2)
                            .unsqueeze(1).broadcast_to([128, 8, 2, 32]))
                        nc.vector.tensor_add(q1[:], q1[:], q2[:])
                        return q1

                    def q_stage2(st, hf, qr, msq, qTps):
                        lnq = tA.tile([128, 8], F32, tag="lnq", name=f"lnq{st}{hf}")
                        nc.scalar.activation(lnq[:], msq[:], AF.Ln, bias=epsc[:],
                                             scale=1.0 / D)
                        invq = tA.tile([128, 8], F32, tag="invq", name=f"invq{st}{hf}")
                        nc.scalar.activation(invq[:], lnq[:], AF.Exp, scale=-0.5)
                        qrs = tA.tile([128, 512], F32R, tag="qrs", name=f"qrs{st}{hf}")
                        nc.vector.tensor_mul(
                            qrs[:].rearrange("p (h d) -> p h d", h=8),
                            qr[:].rearrange("p (h d) -> p h d", h=8),
                            invq[:].unsqueeze(2).broadcast_to([128, 8, D]))
                        j = st - 4
                        for db4 in range(4):
                            sl = slice(db4 * 128, (db4 + 1) * 128)
                            nc.tensor.matmul(qTps[db4][:, j * 128:(j + 1) * 128],
                                             qrs[:, sl], identr[:],
                                             is_transpose=True, start=True, stop=True)
                            nc.scalar.copy(qT_sb[hf * 4 + db4][:, j * 128:(j + 1) * 128],
                                           qTps[db4][:, j * 128:(j + 1) * 128])

                    def q_stage1(st, hf):
                        qp = psmm.tile([128, 512], F32, tag="qp", name=f"qp{st}{hf}")
                        for kb in range(8):
                            nc.tensor.matmul(qp[:], xT_sb[kb][:, st * 128:(st + 1) * 128],
                                             wq_sb[kb][:, hf * 512:(hf + 1) * 512],
                                             start=(kb == 0), stop=(kb == 7))
                        sqq = tA.tile([128, 512], F32, tag="sqq", name=f"sqq{st}{hf}")
                        nc.scalar.activation(sqq[:], qp[:], AF.Square)
                        msq = tA.tile([128, 8], F32, tag="msq", name=f"msq{st}{hf}")
                        nc.vector.tensor_reduce(
                            msq[:], sqq[:].rearrange("p (h d) -> p h d", h=8),
                            axis=mybir.AxisListType.X, op=OP.add)
                        qr = q_rope(st, hf, qp)
                        return qr, msq

                    qTps = [pst.tile([128, 512], F32R, tag=f"qTps{d}",
                                     name=f"qTps{hf}_{d}") for d in range(4)]
                    pend = []
                    for st in range(4, 8):
                        pend.append((st, hf, *q_stage1(st, hf)))
                        if len(pend) > 2:
                            q_stage2(*pend.pop(0), qTps)
                    for it in pend:
                        q_stage2(*it, qTps)

                with tc.tile_pool(name="tAq0", bufs=2) as tA0, \
                     tc.tile_pool(name="psmmq0", bufs=4, space="PSUM") as psmm0, \
                     tc.tile_pool(name="pstq0", bufs=1, space="PSUM") as pst0:
                    emit_A1(0, tA0, psmm0, pst0)

            # ======== phase B: attention;  phase C: out-projection ========
            with tc.tile_pool(name="wB", bufs=1) as wB, \
                 tc.tile_pool(name="sbB", bufs=2) as sbB:
                masks = wB.tile([128, 4 * 256], BF16, tag="masks")
                nc.gpsimd.dma_start(masks[:], maskd)
                sinkrs = []
                for hh in range(2):
                    t = wB.tile([8, 512], F32R, tag=f"sinkr{hh}",
                                name=f"sinkr{hh}")
                    nc.gpsimd.dma_start(t[:], sinkd[hh * 8:(hh + 1) * 8, :])
                    sinkrs.append(t)
                woT_sb = []
                for kb in range(8):
                    t = wB.tile([128, HID], BF16, tag=f"wo{kb}", name=f"wo{kb}")
                    nc.gpsimd.dma_start(t[:], woT[kb * 128:(kb + 1) * 128, :])
                    woT_sb.append(t)

                den_sbs = [wB.tile([8, 512], BF16, tag=f"den_sb{hh}",
                                   name=f"den_sb{hh}") for hh in range(2)]

                def emit_C_group(psC, sblk, nh, kbs, op=None, drain=False):
                    if op is None:
                        op = psC.tile([128, 512], F32, tag="op",
                                      name=f"op{sblk}{nh}")
                    for kb in kbs:
                        nc.tensor.matmul(op[:],
                                         aoT_sb[kb][:, sblk * 128:(sblk + 1) * 128],
                                         woT_sb[kb][:, nh * 512:(nh + 1) * 512],
                                         start=(kb == 0), stop=(kb == 7))
                    if drain:
                        osb = sbB.tile([128, 512], F32, tag="osb")
                        nc.scalar.copy(osb[:], op[:])
                        eng = nc.sync if (sblk + nh) % 2 == 0 else nc.scalar
                        eng.dma_start(
                            outd[sblk * 128:(sblk + 1) * 128,
                                 nh * 512:(nh + 1) * 512], osb[:])
                    return op
                rec_sbs = [wB.tile([8, 512], F32R, tag=f"rec_sb{hh}",
                                   name=f"rec_sb{hh}") for hh in range(2)]
                rec1 = wB.tile([1, 16 * 512], F32R, tag="rec1")
                onesrow = wB.tile([1, 128], F32R, tag="onesrow")
                nc.vector.memset(onesrow[:].bitcast(F32), 1.0)
                if "B" not in phases:
                    for hh in range(2):
                        nc.vector.memset(den_sbs[hh][:], 1.0)
                    for p in range(8):
                        nc.vector.memset(aoT_sb[p][:].bitcast(F32), 0.0)

                def emit_B(plist, psp, psav):
                    for p in plist:
                      kv = p // 2
                      for Q in range(2):
                          psb = sbB.tile([128, 3072], BF16, tag="psb")
                          for h2 in range(2):
                              b = 64 * h2
                              pp = psp.tile([128, 1536], F32, tag="pp")
                              for nu in range(6):
                                  kap = 2 * Q + nu
                                  lhsT = kTd_sb[kv][b:b + 64, kap * 128:(kap + 1) * 128]
                                  rhs = qT_sb[p][b:b + 64, Q * 256:(Q + 1) * 256]
                                  nc.tensor.matmul(pp[:, nu * 256:(nu + 1) * 256],
                                                   lhsT, rhs, start=True, stop=True)
                              nc.scalar.activation(psb[:, h2 * 1536:(h2 + 1) * 1536],
                                                   pp[:], AF.Exp, scale=SCALE)
                              # zero the disallowed positions (binary mask)
                              mv = psb[:, h2 * 1536: (h2 + 1) * 1536] \
                                  .rearrange("p (a c) -> p a c", a=3)[:, 0::2, :]
                              nc.vector.tensor_mul(
                                  mv, mv,
                                  masks[:].rearrange("p (a c) -> p a c", a=2))
                          avp = psav.tile([65, 512], F32, tag="avp")
                          for h2 in range(2):
                              for nu in range(6):
                                  stk = 2 * Q + nu
                                  rhs = psb[:, h2 * 1536 + nu * 256: h2 * 1536 + (nu + 1) * 256]
                                  nc.tensor.matmul(avp[:, h2 * 256:(h2 + 1) * 256],
                                                   v_sb[stk][:, kv * 65:(kv + 1) * 65], rhs,
                                                   start=(nu == 0), stop=(nu == 5))
                          av2 = sbB.tile([65, 512], BF16, tag="av2")
                          nc.vector.tensor_copy(av2[:], avp[:])
                          r = 2 * p + Q
                          nc.sync.dma_start(den_sbs[r // 8][r % 8: r % 8 + 1, :],
                                            av2[64:65, :])
                          nc.sync.dma_start(aoT_sb[p][0:64, Q * 256:(Q + 1) * 256],
                                            av2[0:64, 0:256])
                          nc.sync.dma_start(aoT_sb[p][64:128, Q * 256:(Q + 1) * 256],
                                            av2[0:64, 256:512])

                def emit_norm(half, psrep):
                    # reciprocal + broadcast + normalize for p in [4*half, 4*half+4)
                    r0 = half * 8
                    nc.vector.tensor_add(rec_sbs[half][:].bitcast(F32),
                                         den_sbs[half][:],
                                         sinkrs[half][:].bitcast(F32))
                    nc.vector.reciprocal(rec_sbs[half][:].bitcast(F32),
                                         rec_sbs[half][:].bitcast(F32))
                    nc.sync.dma_start(rec1[0:1, r0 * 512:(r0 + 8) * 512],
                                      rec_sbs[half][:])
                    for p in range(half * 4, half * 4 + 4):
                      for Q in range(2):
                        base = (2 * p + Q) * 512
                        repA = psrep.tile([128, 256], F32, tag="repA")
                        repB = psrep.tile([128, 256], F32, tag="repB")
                        nc.tensor.matmul(repA[:], onesrow[:],
                                         rec1[0:1, base: base + 256])
                        nc.tensor.matmul(repB[:], onesrow[:],
                                         rec1[0:1, base + 256: base + 512])
                        nc.vector.tensor_mul(aoT_sb[p][0:64, Q * 256:(Q + 1) * 256],
                                             aoT_sb[p][0:64, Q * 256:(Q + 1) * 256],
                                             repA[0:64, :])
                        nc.vector.tensor_mul(aoT_sb[p][64:128, Q * 256:(Q + 1) * 256],
                                             aoT_sb[p][64:128, Q * 256:(Q + 1) * 256],
                                             repB[64:128, :])

                if "B" in phases:
                    with tc.tile_pool(name="psp0", bufs=2, space="PSUM") as psp, \
                         tc.tile_pool(name="psav0", bufs=2, space="PSUM") as psav:
                        emit_B([0, 1, 2, 3], psp, psav)
                    with tc.tile_pool(name="tAq1", bufs=2) as tA1, \
                         tc.tile_pool(name="psmmq1", bufs=4, space="PSUM") as psmm1, \
                         tc.tile_pool(name="pstq1", bufs=1, space="PSUM") as pst1:
                        emit_A1(1, tA1, psmm1, pst1)
                    with tc.tile_pool(name="psrep0", bufs=2, space="PSUM") as psrep:
                        emit_norm(0, psrep)
                    with tc.tile_pool(name="psp1", bufs=2, space="PSUM") as psp, \
                         tc.tile_pool(name="psav1", bufs=2, space="PSUM") as psav:
                        emit_B([4, 5, 6, 7], psp, psav)
                    with tc.tile_pool(name="psC", bufs=4, space="PSUM") as psC:
                        # start sblk 0-1 with the already-normalized first-half
                        # heads while the second half's denominators settle
                        held = {}
                        for sblk in (0, 1):
                            for nh in range(2):
                                held[(sblk, nh)] = emit_C_group(
                                    psC, sblk, nh, range(4))
                        with tc.tile_pool(name="psrep1", bufs=2,
                                          space="PSUM") as psrep:
                            emit_norm(1, psrep)
                        for sblk in (0, 1):
                            for nh in range(2):
                                emit_C_group(psC, sblk, nh, range(4, 8),
                                             op=held[(sblk, nh)], drain=True)
                        for sblk in (2, 3):
                            for nh in range(2):
                                emit_C_group(psC, sblk, nh, range(8), drain=True)
                else:
                    with tc.tile_pool(name="tAq1", bufs=2) as tA1, \
                         tc.tile_pool(name="psmmq1", bufs=4, space="PSUM") as psmm1, \
                         tc.tile_pool(name="pstq1", bufs=1, space="PSUM") as pst1:
                        emit_A1(1, tA1, psmm1, pst1)
                    with tc.tile_pool(name="psC2", bufs=2, space="PSUM") as psC:
                        for sblk in (range(4) if "C" in phases else []):
                            for nh in range(2):
                                emit_C_group(psC, sblk, nh, range(8), drain=True)



    nc.compile()
    return nc


def _prep_inputs(x, cos, sin, wq, wk, wv, wo, q_norm_w, k_norm_w, sinks):
    """Build the 8 per-core input maps."""
    x = np.asarray(x, np.float32).reshape(S, HID)
    cos = np.asarray(cos, np.float32)
    sin = np.asarray(sin, np.float32)
    wq = np.asarray(wq, np.float32)
    wk = np.asarray(wk, np.float32)
    wv = np.asarray(wv, np.float32)
    wo = np.asarray(wo, np.float32)
    qw = np.asarray(q_norm_w, np.float32)
    kw = np.asarray(k_norm_w, np.float32)
    sinks = np.asarray(sinks, np.float32)

    wqT = np.ascontiguousarray(wq.T)                      # [HID, H*D]
    wkvT = np.ascontiguousarray(np.concatenate([wk, wv], 0).T)  # [HID, 512]
    woT = np.ascontiguousarray(wo.T)                      # [H*D, HID]
    ident = np.eye(128, dtype=np.float32)

    # rope coefficient tables with norm weight folded in
    # q'' [d] = qn[d]*w[d]*cos[d] + rot(qn*w)[d]*sin[d]
    #   rot(qn*w)[d<32] = -qn[d+32]*w[d+32]; rot[d>=32] = qn[d-32]*w[d-32]
    sgn = np.concatenate([-np.ones(32, np.float32), np.ones(32, np.float32)])
    wrot_q = np.concatenate([qw[32:], qw[:32]])
    wrot_k = np.concatenate([kw[32:], kw[:32]])
    cw_q = cos * qw[None, :]
    sw_q = sin * (sgn * wrot_q)[None, :]
    cw_k = cos * kw[None, :]
    sw_k = sin * (sgn * wrot_k)[None, :]

    # additive masks for partial nu blocks (order nu=0,1,4,5)
    r = np.arange(128)[:, None]
    c = np.arange(256)[None, :]
    mstack = []
    for nu in (0, 1, 4, 5):
        ij = c - r + 512 - 128 * nu
        allowed = (ij >= 0) & (ij < WINDOW)
        mstack.append(np.where(allowed, 1.0, 0.0).astype(np.float32))
    masks = np.concatenate(mstack, 1)                     # [128, 1024]

    xT = np.ascontiguousarray(x.T)                        # [HID, S]
    esink = np.exp(sinks.astype(np.float64)).astype(np.float32)

    in_maps = []
    for core in range(NCORE):
        start = SLOC * core
        lo = start - WINDOW
        xt_loc = np.zeros((HID, SKV), np.float32)
        srclo = max(0, lo)
        xt_loc[:, srclo - lo:] = xT[:, srclo:start + SLOC]
        idx_k = np.clip(np.arange(lo, start + SLOC), 0, S - 1)
        # sink rhs: row 2p+Q, col h2*256+qq -> exp(sink[2p+h2]).
        # core 0: halo keys' denominator contributions are suppressed by
        # zeroing their v ones-column (vones), as large partial sums would
        # hit the reduced-precision psum accumulate.
        sink_rhs = np.zeros((16, 512), np.float32)
        for p in range(8):
            for Qb in range(2):
                for h2 in range(2):
                    sink_rhs[2 * p + Qb, h2 * 256:(h2 + 1) * 256] = esink[2 * p + h2]
        vones = np.ones((SKV, 1), np.float32)
        if core == 0:
            vones[:WINDOW] = 0.0
        # packed rope tables: [128, st-blocks*64] (cos blocks then sin blocks)
        cq_loc = cw_q[start:start + SLOC].reshape(4, 128, 64)
        sq_loc = sw_q[start:start + SLOC].reshape(4, 128, 64)
        cqsq = np.concatenate(
            [cq_loc[i] for i in range(4)] + [sq_loc[i] for i in range(4)], axis=1)
        ck_loc = cw_k[idx_k].reshape(8, 128, 64)
        sk_loc = sw_k[idx_k].reshape(8, 128, 64)
        cksk = np.concatenate(
            [ck_loc[i] for i in range(8)] + [sk_loc[i] for i in range(8)], axis=1)
        in_maps.append(dict(
            xT=xt_loc.astype(BF16NP),
            wqT=wqT.astype(BF16NP), wkvT=wkvT.astype(BF16NP),
            woT=woT.astype(BF16NP),
            cqsq=np.ascontiguousarray(cqsq),
            cksk=np.ascontiguousarray(cksk),
            masks=masks.astype(BF16NP), ident=ident.astype(BF16NP),
            identr=ident,
            sink_rhs=sink_rhs, vones=vones.astype(BF16NP),
        ))
    return in_maps


def kernel(x, cos, sin, wq, wk, wv, wo, q_norm_w, k_norm_w, sinks, **kw):
    if "nc" not in _cache:
        _cache["nc"] = _build()
    nc = _cache["nc"]
    in_maps = _prep_inputs(x, cos, sin, wq, wk, wv, wo, q_norm_w, k_norm_w, sinks)
    res = run_bass_kernel_spmd(nc, in_maps, core_ids=list(range(NCORE)), **kw)
    out = np.empty((S, HID), np.float32)
    for core in range(NCORE):
        out[core * SLOC:(core + 1) * SLOC] = res.results[core]["out"]
    if kw:
        _cache["last_results"] = res
    return out.reshape(B, S, HID)
